# revision 32
# baseline (speedup 1.0000x reference)
"""Trainium2 Bass kernel for Swin-style window attention (fp16 datapath).

Problem: x[256,196,768] -> window attention (12 heads, d=64, relative
position bias gathered from a 729x12 table) -> out[256,196,768], fp32.

Sharding: data-parallel over the 256 windows across 8 NeuronCores
(32 windows per core), window PAIRS per step (N=392 free dim for the
dense matmuls).  Weights + bias table replicated to every core.

Why fp16: PE streams fp16 at 1 cyc/row at ANY free size (f32r needs
free>=256 and the AV matmuls only have 196), fp16 carries 10 mantissa
bits (f32r-level precision, ~5e-4 end-to-end), DMA bytes halve, and
no on-device f32->f32r rounding copies are needed - DMA lands fp16
tiles that matmuls read directly.

Device dataflow per pair (xT[c, tok] layout, tokens free):
  - qk psum[mt] = qk_w.T @ x          mt 0..5 = q (pre-scaled 1/8),
                                      6..11 = k; copied to fp16 sbuf
  - v[j, c'] = x.T @ v_w              per (s, jt); copied to fp16
                                      v_sb[j, h, (d'|ones64)] so AV's
                                      psum rows 64:128 give softmax
                                      denominators
  - attn per head-pair: ap2[j, jt, (s,i)] = k_h.T @ q_h - 8 interleaved
    K=64 matmuls; even/odd heads auto-land on PE row groups 0/64 and
    run concurrently in the array; each head's logits live in ONE
    2-bank psum tile so exp and the bias multiply run once per head
    at free size 784
  - ET = exp(ap2) -> fp16 (ACT), et = ET * exp(bias) (column-split
    across the otherwise-idle gpsimd engine and DVE)
  - av2[(d'|ones), he, (s,i)] = [v|1].T @ et   (K=128 + K=68; both
    heads of the pair share one 2-bank psum tile -> one ACT staging
    copy + one DVE reciprocal per head-pair)
  - aoT[c-dims of h] = av2[0:64] * rbs -> fp16
  - proj: out psum[mt] = p_w.T @ aoT -> f32 copy (ACT/DVE) -> DMA out
Emission is software-pipelined: attention runs SKEW head-pairs behind
the qk projection; the last head-pair + projection of each pair are
deferred into the next pair's slot AFTER its v matmuls (v_sb is
double-buffered), so the pair boundary has dense PE work while the
tail's exp/AV/reciprocal chain drains at high queue priority.
"""

import os
from contextlib import ExitStack

import numpy as np

import concourse.bacc as bacc
import concourse.mybir as mybir
import concourse.tile as tile
from concourse import bass_utils

# ---- problem constants (hardcoded per harness contract) ----
B = 256          # windows total
NCORES = 8
NW = B // NCORES  # 32 windows per core
NTOK = 196
C = 768
H = 12
D = 64
KK = C // 128    # 6 contraction tiles
NPAIR_FULL = NW // 2   # 16 window pairs per core
NPF = 2 * NTOK   # 392 tokens per pair
J0 = 128         # first j tile rows
J1 = NTOK - 128  # 68

F32 = mybir.dt.float32
F16 = mybir.dt.float16

_PROGRAM_CACHE = {}


def _build_program(n_pairs):
    nc = bacc.Bacc("TRN2", target_bir_lowering=False, debug=False)

    xT_d = nc.dram_tensor("xT", [C, NW * NTOK], F16, kind="ExternalInput")
    qk_w_d = nc.dram_tensor("qk_w", [C, 2 * C], F16, kind="ExternalInput")
    v_w_d = nc.dram_tensor("v_w", [C, C], F16, kind="ExternalInput")
    p_w_d = nc.dram_tensor("p_w", [C, C], F16, kind="ExternalInput")
    # bias exp table, [j-part, h, jt, (s,i)] with jt=0 -> j 0:128,
    # jt=1 -> j 128:196 (rows 68:128 of that slice unused)
    b_d = nc.dram_tensor("b", [J0, H, 2, NPF], F16, kind="ExternalInput")
    outT_d = nc.dram_tensor("outT", [C, NW * NTOK], F32, kind="ExternalOutput")

    with tile.TileContext(nc) as tc, ExitStack() as ctx:
        const = ctx.enter_context(tc.tile_pool(name="const", bufs=1))
        xsp = ctx.enter_context(tc.tile_pool(name="xsp", bufs=3))
        qkp = ctx.enter_context(tc.tile_pool(name="qkp", bufs=1))
        vp = ctx.enter_context(tc.tile_pool(name="vp", bufs=2))
        aop = ctx.enter_context(tc.tile_pool(name="aop", bufs=2))
        etp = ctx.enter_context(tc.tile_pool(name="etp", bufs=4))
        small = ctx.enter_context(tc.tile_pool(name="small", bufs=2))
        otp = ctx.enter_context(tc.tile_pool(name="otp", bufs=2))
        ps_big = ctx.enter_context(tc.tile_pool(name="ps_big", bufs=2, space="PSUM"))
        ps_attn = ctx.enter_context(tc.tile_pool(name="ps_attn", bufs=1, space="PSUM"))
        ps_av = ctx.enter_context(tc.tile_pool(name="ps_av", bufs=2, space="PSUM"))

        # ---- pair-0 x load first so compute can start ASAP ----
        # per-kk tiles: consumers depend on single DMAs, not all six
        xs0 = [xsp.tile([128, NPF], F16, tag=f"xs{kk}", name=f"xs{kk}")
               for kk in range(KK)]
        for kk in range(KK):
            nc.sync.dma_start(xs0[kk][:], xT_d[kk * 128:(kk + 1) * 128, 0:NPF])

        # ---- one-time: weights (fp16, no staging copies) ----
        # v_w first (v matmuls run first in each pair), then qk, bias, proj.
        v_w_s = [const.tile([128, C], F16, tag=f"vw{kk}", name=f"vw{kk}")
                 for kk in range(KK)]
        for kk in range(KK):
            nc.sync.dma_start(v_w_s[kk][:], v_w_d[kk * 128:(kk + 1) * 128, :])
        qk_w_s = [const.tile([128, 2 * C], F16, tag=f"qkw{kk}", name=f"qkw{kk}")
                  for kk in range(KK)]
        for kk in range(KK):
            nc.sync.dma_start(qk_w_s[kk][:], qk_w_d[kk * 128:(kk + 1) * 128, :])
        wb = const.tile([J0, H, 2, NPF], F16, tag="wb")
        nc.sync.dma_start(wb[:], b_d[:])
        p_w_s = [const.tile([128, C], F16, tag=f"pw{kk}", name=f"pw{kk}")
                 for kk in range(KK)]
        for kk in range(KK):
            nc.sync.dma_start(p_w_s[kk][:], p_w_d[kk * 128:(kk + 1) * 128, :])

        # prime the ACT exp table early
        dummy = small.tile([1, 16], F32, tag="dummy")
        nc.gpsimd.memset(dummy[:], 0.0)
        nc.scalar.activation(dummy[:], dummy[:],
                             mybir.ActivationFunctionType.Exp)

        def emit_attn_pair(hp, q_sb, k_sb, v_sb, aoT, hot=False):
            if hot:
                # tail head-pair: its exp/mult/AV/recip chain gates the
                # deferred projection - let it win engine-queue ties
                with tc.high_priority():
                    _emit_attn_pair(hp, q_sb, k_sb, v_sb, aoT)
            else:
                _emit_attn_pair(hp, q_sb, k_sb, v_sb, aoT)

        def _emit_attn_pair(hp, q_sb, k_sb, v_sb, aoT):
            # Both heads of the pair: interleaved K=64 QK matmuls (row
            # groups 0/64 run concurrently), one 2-bank psum per head so
            # exp and the bias multiply run once per head at free=784.
            ap2 = []
            for he in (0, 1):
                ap2.append(ps_attn.tile([128, 2, 512], F32, tag="ap2",
                                        name="ap2"))
            for jt, jsz in ((0, J0), (1, J1)):
                for s in (0, 1):
                    for he in (0, 1):
                        ho = he * 64
                        nc.tensor.matmul(
                            ap2[he][:jsz, jt, s * NTOK:(s + 1) * NTOK],
                            k_sb[hp][ho:ho + 64,
                                     s * NTOK + jt * 128:
                                     s * NTOK + jt * 128 + jsz],
                            q_sb[hp][ho:ho + 64, s * NTOK:(s + 1) * NTOK],
                            start=True, stop=True)
            ets = []
            for he in (0, 1):
                h = 2 * hp + he
                eraw = etp.tile([128, 2, NPF], F16, tag="eraw")
                nc.scalar.activation(eraw[:], ap2[he][:, :, 0:NPF],
                                     mybir.ActivationFunctionType.Exp)
                et = etp.tile([128, 2, NPF], F16, tag="et")
                # column-split across the idle gpsimd engine and DVE
                # (gpsimd is ~3x slower per element; keep its share small
                # so the et -> AV chain latency stays low)
                nc.gpsimd.tensor_mul(et[:, :, 0:96], eraw[:, :, 0:96],
                                     wb[:, h, :, 0:96])
                nc.vector.tensor_mul(et[:, :, 96:], eraw[:, :, 96:],
                                     wb[:, h, :, 96:])
                ets.append(et)
            av2 = ps_av.tile([128, 2, 512], F32, tag="av2")
            for he in (0, 1):
                h = 2 * hp + he
                for s in (0, 1):
                    for jt, jsz in ((0, J0), (1, J1)):
                        nc.tensor.matmul(
                            av2[:, he, s * NTOK:(s + 1) * NTOK],
                            v_sb[s][jt][:jsz, h],
                            ets[he][:jsz, jt, s * NTOK:(s + 1) * NTOK],
                            start=(jt == 0), stop=(jt == 1))
            s_sb = small.tile([64, 2, NPF], F32, tag="s_sb")
            nc.scalar.copy(s_sb[:], av2[64:128, :, 0:NPF])
            rbs = small.tile([64, 2, NPF], F32, tag="rbs")
            nc.vector.reciprocal_approx_fast(rbs[:], s_sb[:])
            for he in (0, 1):
                nc.vector.tensor_tensor(
                    aoT[he * 64:he * 64 + 64, hp, :],
                    av2[0:64, he, 0:NPF], rbs[:, he, :],
                    op=mybir.AluOpType.mult)

        def emit_proj(aoT, ncol):
            for mt in range(6):
                p = ps_big.tile([128, NPF], F32, tag="big")
                for kk in range(KK):
                    nc.tensor.matmul(
                        p[:], p_w_s[kk][:, mt * 128:(mt + 1) * 128],
                        aoT[:, kk, :], start=(kk == 0), stop=(kk == KK - 1))
                ot = otp.tile([128, NPF], F32, tag="ot")
                if mt % 2 == 0:
                    nc.scalar.copy(ot[:], p[:])
                else:
                    nc.vector.tensor_copy(ot[:], p[:])
                nc.sync.dma_start(
                    outT_d[mt * 128:(mt + 1) * 128, ncol], ot[:])

        SKEW = int(os.environ.get("BASS_SKEW", "2"))
        pending = []  # deferred attention heads + proj of previous pair
        for pair in range(n_pairs):
            ncol = slice(pair * NPF, (pair + 1) * NPF)
            # ---- load xT for this pair (fp16, matmul-ready) ----
            if pair == 0:
                xs = xs0
            else:
                xs = [xsp.tile([128, NPF], F16, tag=f"xs{kk}",
                               name=f"xs{kk}") for kk in range(KK)]
                for kk in range(KK):
                    nc.sync.dma_start(
                        xs[kk][:], xT_d[kk * 128:(kk + 1) * 128, ncol])

            # ---- v: per (s, jt): [j, 768] via lhsT = xT tiles ----
            # (the deferred tail of the previous pair is flushed AFTER the
            # v matmuls: v_sb is double-buffered, so v starts right at the
            # pair boundary and the tail's exp/AV chain hides under it)
            v_sb = []
            for s in (0, 1):
                vs = []
                for jt, jsz in ((0, J0), (1, J1)):
                    vt = vp.tile([128, H, 128], F16, tag=f"v{s}{jt}")
                    if pair < 2:
                        # ones columns 64:128 for the denominator rows
                        # (first use of each of the 2 rotating buffers)
                        nc.gpsimd.memset(vt[:jsz, :, 64:128], 1.0)
                    for n in (0, 1):
                        p = ps_big.tile([128, 384], F32, tag="big")
                        for kk in range(KK):
                            nc.tensor.matmul(
                                p[:jsz],
                                xs[kk][:,
                                       s * NTOK + jt * 128:
                                       s * NTOK + jt * 128 + jsz],
                                v_w_s[kk][:, n * 384:(n + 1) * 384],
                                start=(kk == 0), stop=(kk == KK - 1))
                        dst = vt[:jsz, n * 6:(n + 1) * 6, 0:64]
                        src = p[:jsz].rearrange("p (h d) -> p h d", d=64)
                        if (s + jt) % 2 == 0:
                            nc.vector.tensor_copy(dst, src)
                        else:
                            nc.scalar.copy(dst, src)
                    vs.append(vt)
                v_sb.append(vs)

            # flush deferred tail of previous pair (last heads + proj)
            for fn in pending:
                fn()
            pending = []

            aoT = aop.tile([128, KK, NPF], F16, tag="aoT")
            q_sb = {}
            k_sb = {}

            queue = []  # head-pairs whose attention is not yet emitted
            for hp in range(6):
                for half, dst in ((0, q_sb), (1, k_sb)):
                    p = ps_big.tile([128, NPF], F32, tag="big")
                    off = half * C + hp * 128
                    for kk in range(KK):
                        nc.tensor.matmul(
                            p[:], qk_w_s[kk][:, off:off + 128],
                            xs[kk][:], start=(kk == 0), stop=(kk == KK - 1))
                    t = qkp.tile([128, NPF], F16, tag=f"{'qk'[half]}{hp}")
                    if half == 0:
                        nc.scalar.copy(t[:], p[:])
                    else:
                        nc.vector.tensor_copy(t[:], p[:])
                    dst[hp] = t

                queue.append(hp)
                if len(queue) > SKEW:
                    php = queue.pop(0)
                    emit_attn_pair(php, q_sb, k_sb, v_sb, aoT)

            # defer remaining head-pairs + proj into the next pair's slot
            def tail(queue=queue, q_sb=q_sb, k_sb=k_sb, v_sb=v_sb,
                     aoT=aoT, ncol=ncol):
                for php in queue:
                    emit_attn_pair(php, q_sb, k_sb, v_sb, aoT, hot=True)
                emit_proj(aoT, ncol)
            pending = [tail]

        for fn in pending:
            fn()

    nc.compile()
    return nc


def kernel(**inputs):
    x = np.asarray(inputs["x"], dtype=np.float32)
    qkv_w = np.asarray(inputs["qkv_w"], dtype=np.float32)
    qkv_b = np.asarray(inputs["qkv_b"], dtype=np.float32)
    proj_w = np.asarray(inputs["proj_w"], dtype=np.float32)
    proj_b = np.asarray(inputs["proj_b"], dtype=np.float32)
    rpe_table = np.asarray(inputs["rpe_table"], dtype=np.float32)
    rpe_index = np.asarray(inputs["rpe_index"])

    if np.any(qkv_b[:C]):
        raise NotImplementedError("nonzero q bias not supported")
    # k bias shifts logits by a per-i constant -> softmax invariant.
    # v bias and proj bias are exact host-side output folds (see below).

    qk_w = qkv_w[:, :2 * C].copy()
    qk_w[:, :C] *= (1.0 / np.sqrt(D))
    qk_w16 = qk_w.astype(np.float16)
    v_w16 = np.ascontiguousarray(qkv_w[:, 2 * C:]).astype(np.float16)
    p_w16 = proj_w.astype(np.float16)

    bias = rpe_table[np.asarray(rpe_index, np.int64).reshape(-1)]
    bias = bias.reshape(NTOK, NTOK, H)
    wexp = np.exp(bias).transpose(1, 2, 0)          # [j, h, i]
    wb2 = np.concatenate([wexp, wexp], axis=2)       # [j, h, (s0 i|s1 i)]
    b = np.zeros((J0, H, 2, NPF), np.float16)
    b[:, :, 0, :] = wb2[:J0]
    b[:J1, :, 1, :] = wb2[J0:]
    n_pairs = int(os.environ.get("BASS_NPAIRS", NPAIR_FULL))
    if n_pairs not in _PROGRAM_CACHE:
        _PROGRAM_CACHE[n_pairs] = _build_program(n_pairs)
    nc = _PROGRAM_CACHE[n_pairs]

    shared = {
        "qk_w": qk_w16, "v_w": v_w16, "p_w": p_w16,
        "b": b,
    }
    in_maps = []
    for c in range(NCORES):
        xT = np.ascontiguousarray(
            x[c * NW:(c + 1) * NW].transpose(2, 0, 1).reshape(
                C, NW * NTOK).astype(np.float16))
        in_maps.append({"xT": xT, **shared})

    trace = bool(int(os.environ.get("BASS_KERNEL_TRACE", "0")))
    res = bass_utils.run_bass_kernel_spmd(
        nc, in_maps, core_ids=list(range(NCORES)), trace=trace)
    kernel.last_result = res

    out = np.empty((B, NTOK, C), np.float32)
    for c in range(NCORES):
        outT = res.results[c]["outT"]
        out[c * NW:(c + 1) * NW] = outT.reshape(
            C, NW, NTOK).transpose(1, 2, 0)

    if np.any(qkv_b[2 * C:]) or np.any(proj_b):
        out += qkv_b[2 * C:] @ proj_w + proj_b
    return out
